# revision 12
# baseline (speedup 1.0000x reference)
"""DiT block kernel for 8 Trainium2 NeuronCores.

Data-parallel over batch (2 per core). All big GEMMs run fp8e4m3 with
DoubleRow perf mode. Power-of-2 scale factors keep fp8 tensors in the e4m3
sweet spot; descales fold into existing ACT/DVE ops.

v2 restructure: the softmax exp stream on the ACT engine paces attention
(~9.2us/head) while the PE only has ~6.2us/head of score/av matmuls; the
v1 kernel let the PE idle in ~3us gaps which re-throttled the HAM clock
gate (K=4/8, 1.2GHz) for the whole attention region. Here the v GEMM, the
b1 half of qk-norm, and proj(b0) are emitted as filler units between score
blocks so the PE stays saturated (and warm) through attention. PSUM->SBUF
copies moved off ACT onto DVE so ACT runs pure exp.
"""

import sys

sys.path.insert(0, "/opt/trn_rl_repo")

from collections import deque
from contextlib import ExitStack

import ml_dtypes
import numpy as np

import concourse.bacc as bacc
import concourse.tile as tile
from concourse import mybir
from concourse.bass_utils import run_bass_kernel_spmd

FP32 = mybir.dt.float32
FP32R = mybir.dt.float32r
FP8 = mybir.dt.float8e4
BF16 = mybir.dt.bfloat16
AF = mybir.ActivationFunctionType
ALU = mybir.AluOpType
DRM = mybir.MatmulPerfMode.DoubleRow
E4 = ml_dtypes.float8_e4m3

B, N, H = 16, 1024, 1152
NH, HD = 16, 72
MLP = H * 4
NCORES = 8
BPC = B // NCORES            # 2
T = BPC * N                  # 2048
NTC = T // 512               # 4
FK = H // 128                # 9
FKP = 10                     # padded K planes for H-contractions
QKF = (2 * H) // 128         # 18
MK = MLP // 128              # 36
EPS = 1e-6
ISQ = float(HD) ** -0.5
EXPB = -2.5
VCOL = 97                    # z ones-row at partition 96 (32-aligned for DVE)

# power-of-2 fp8 scales
SXN = 16.0
SWQK = 4096.0
SQ = 2.0  # keep (SQ*q)^2 well under the e4m3 448 max
SQN = 16.0
SWV = 4096.0
SV = 16.0
SO = 16.0
SWPRO = 2048.0
SW1 = 4096.0
SW2 = 4096.0
SMOD = 1.0   # mod path runs bf16: no range scaling needed
SSW = 1.0
DMOD = 1.0 / (SMOD * SSW)
CQ = SQ / (SWQK * SXN)
CV = SV / (SWV * SXN)
CPR = 1.0 / (SWPRO * SO)
CM2 = 1.0 / SW2
CEXP = ISQ / (SQN * SQN)
CG1 = 1.0 / (SW1 * SXN)
assert SO == SV  # rzp expansion row uses plain ones

_CACHE = {}
SIM_COMPAT = False  # decompose Silu/Gelu for CoreSim (no LUTs there)
VBLK = ((0, 6), (6, 12), (12, 16))  # head-group blocks for the v matmul


def _build_program():
    nc = bacc.Bacc("TRN2", target_bir_lowering=False, debug=False)

    xT = nc.dram_tensor("xT", [H, T], FP32R, kind="ExternalInput")
    cT = nc.dram_tensor("cT", [H, BPC], FP32, kind="ExternalInput")
    wmod8 = nc.dram_tensor("wmod8", [128, 18 * FK * 384], BF16, kind="ExternalInput")
    bmod_s = nc.dram_tensor("bmod_s", [1, 6 * H], BF16, kind="ExternalInput")
    wqk8 = nc.dram_tensor("wqk8", [128, QKF * FKP * 128], FP8, kind="ExternalInput")
    wv8 = nc.dram_tensor("wv8", [128, FKP * H], FP8, kind="ExternalInput")
    bqk = nc.dram_tensor("bqk", [128, QKF], FP32, kind="ExternalInput")
    exr = nc.dram_tensor("exr", [32, QKF * 128], BF16, kind="ExternalInput")
    ind8s = nc.dram_tensor("ind8s", [128, QKF * 64], FP8, kind="ExternalInput")
    ind8q = nc.dram_tensor("ind8q", [128, QKF * 64], FP8, kind="ExternalInput")
    wpro8 = nc.dram_tensor("wpro8", [128, FK * FKP * 128], FP8, kind="ExternalInput")
    w18 = nc.dram_tensor("w18", [128, MK * FKP * 128], FP8, kind="ExternalInput")
    w28 = nc.dram_tensor("w28", [128, FK * MK * 128], FP8, kind="ExternalInput")
    onesr = nc.dram_tensor("onesr", [1, 512], FP32R, kind="ExternalInput")
    onesb = nc.dram_tensor("onesb", [1, 512], BF16, kind="ExternalInput")
    onesc = nc.dram_tensor("onesc", [128, 1], FP32R, kind="ExternalInput")
    epsc = nc.dram_tensor("epsc", [128, 4], FP32, kind="ExternalInput")
    out = nc.dram_tensor("out", [H, T], FP32, kind="ExternalOutput")

    with nc.allow_low_precision(
        reason="fp8 matmuls; adaLN gates damp branch error"
    ), tile.TileContext(nc) as tc, ExitStack() as top:
        dram = top.enter_context(tc.tile_pool(name="dram", bufs=1, space="DRAM"))
        qk_d8 = dram.tile([2 * H, T], FP8, tag="qk_d8", name="qk_d8")
        rows_d = dram.tile([6, BPC * H], BF16, tag="rows_d", name="rows_d")

        cst = top.enter_context(tc.tile_pool(name="cst", bufs=1))
        ones512 = cst.tile([1, 512], FP32R, tag="o512", name="o512")
        nc.sync.dma_start(ones512[:], onesr.ap())
        ones16 = cst.tile([1, 512], BF16, tag="o16", name="o16")
        nc.sync.dma_start(ones16[:], onesb.ap())
        ocol128 = cst.tile([128, 1], FP32R, tag="oc128", name="oc128")
        nc.sync.dma_start(ocol128[:], onesc.ap())
        ocol16 = cst.tile([128, 1], BF16, tag="oc16", name="oc16")
        nc.gpsimd.memset(ocol16[:], 1.0)
        eps_t = cst.tile([128, 4], FP32, tag="epsc", name="epsc")
        nc.sync.dma_start(eps_t[:], epsc.ap())
        bqk_t = cst.tile([128, QKF], FP32, tag="bqk", name="bqk")
        nc.sync.dma_start(bqk_t[:], bqk.ap())

        mod_p = top.enter_context(tc.tile_pool(name="modp", bufs=1))
        sc_col = [[mod_p.tile([128, FK], FP32, tag=f"scc{u}{b}", name=f"scc{u}{b}")
                   for b in range(BPC)] for u in range(2)]
        sh_col = [[mod_p.tile([128, FK], FP32, tag=f"shc{u}{b}", name=f"shc{u}{b}")
                   for b in range(BPC)] for u in range(2)]
        g_col = [[mod_p.tile([128, FK], FP32, tag=f"gc{u}{b}", name=f"gc{u}{b}")
                  for b in range(BPC)] for u in range(2)]

        # ---------------- LayerNorm helpers ----------------
        def ln_stats_n(src_of, n, ps, sb, ocol, sqdt, stA16, stC16):
            nsl = slice(n * 512, (n + 1) * 512)
            ln_s = ps.tile([1, 512], FP32, tag="lns", name="lns")
            ln_q = ps.tile([1, 512], FP32, tag="lnq", name="lnq")
            for k in range(FK):
                sq = sb.tile([128, 512], sqdt, tag="sq", name="sq", bufs=2)
                nc.vector.tensor_mul(sq[:], src_of(k)[:, nsl], src_of(k)[:, nsl])
                nc.tensor.matmul(ln_s[:], ocol[:], src_of(k)[:, nsl],
                                 start=(k == 0), stop=(k == FK - 1))
                nc.tensor.matmul(ln_q[:], ocol[:], sq[:],
                                 start=(k == 0), stop=(k == FK - 1))
            ms = sb.tile([1, 512], FP32, tag="ms", name="ms", bufs=1)
            nc.scalar.mul(ms[:], ln_s[:], 1.0 / H)
            msq = sb.tile([1, 512], FP32, tag="msq", name="msq", bufs=1)
            nc.scalar.mul(msq[:], ln_q[:], 1.0 / H)
            m2 = sb.tile([1, 512], FP32, tag="sc4", name="m2", bufs=2)
            nc.vector.tensor_mul(m2[:], ms[:], ms[:])
            var = sb.tile([1, 512], FP32, tag="sc4", name="var", bufs=2)
            nc.vector.tensor_sub(var[:], msq[:], m2[:])
            sd = sb.tile([1, 512], FP32, tag="sc4", name="sd", bufs=2)
            nc.scalar.activation(sd[:], var[:], AF.Sqrt, bias=eps_t[0:1, 0:1],
                                 scale=1.0)
            stA_ = sb.tile([1, 512], FP32, tag="sc4", name="stAf", bufs=2)
            nc.vector.reciprocal_approx_fast(stA_[:], sd[:])
            nc.vector.tensor_copy(stA16[n][:], stA_[:])
            nc.vector.scalar_tensor_tensor(stC16[n][:], ms[:], -1.0, stA_[:],
                                           op0=ALU.mult, op1=ALU.mult)

        def ln_apply_b(src_of, u, dst8, sb, pe, stA16, stC16, b):
                wsl = slice(b * N, (b + 1) * N)
                bcA = pe.tile([128, 2, 512], FP32, tag="bcA", name="bcA")
                bcC = pe.tile([128, 2, 512], FP32, tag="bcC", name="bcC")
                for i in range(2):
                    nc.tensor.matmul(bcA[:, i, :], ones16[0:1, 0:128],
                                     stA16[2 * b + i][:], start=True, stop=True)
                    nc.tensor.matmul(bcC[:, i, :], ones16[0:1, 0:128],
                                     stC16[2 * b + i][:], start=True, stop=True)
                for k in range(FK):
                    y = sb.tile([128, 1024], BF16, tag="y", name="y", bufs=2)
                    nc.vector.tensor_tensor(y[:], src_of(k)[:, wsl], bcA[:, :, :],
                                            op=ALU.mult)
                    y2 = sb.tile([128, 1024], BF16, tag="y2", name="y2", bufs=2)
                    nc.vector.tensor_tensor(y2[:], y[:], bcC[:, :, :], op=ALU.add)
                    nc.gpsimd.tensor_scalar(dst8[:, k, wsl], y2[:],
                                            sc_col[u][b][:, k:k + 1],
                                            sh_col[u][b][:, k:k + 1],
                                            op0=ALU.mult, op1=ALU.add)

        # ------- persistent big tiles (left-stack order = reverse release) ---
        o_p = tc.alloc_tile_pool(name="op", bufs=1)
        o_sb = o_p.tile([128, FKP, T], FP8, tag="o_sb", name="o_sb")
        v_ps = [None, None]
        v_sb = [None, None]
        for b in (1, 0):
            v_ps[b] = tc.alloc_tile_pool(name=f"vp{b}", bufs=1)
            v_sb[b] = v_ps[b].tile([128, 8, NH, VCOL], FP8, tag=f"vsb{b}",
                                   name=f"vsb{b}")
        vw_p = tc.alloc_tile_pool(name="vwp", bufs=1)
        wv_t = vw_p.tile([128, FKP, H], FP8, tag="wv", name="wv")
        xn_p = tc.alloc_tile_pool(name="xnp", bufs=1)
        xn8 = xn_p.tile([128, FKP, T], FP8, tag="xn8", name="xn8")
        nc.vector.memset(xn8[:, FK:FKP, :], 0.0)

        # ---------------- phase 1a: x loads + LN1 stats (PE busy early) ----
        xc_p = tc.alloc_tile_pool(name="xcp", bufs=1, side="right")
        xc = xc_p.tile([128, FK, T], FP32R, tag="xc", name="xc")
        for k in range(FK):
            nc.sync.dma_start(xc[:, k, :], xT.ap()[k * 128:(k + 1) * 128, :])
        src1 = lambda k: xc[:, k, :]
        ln1_p = tc.alloc_tile_pool(name="ln1p", bufs=1, side="right")
        stA16_1 = [ln1_p.tile([1, 512], BF16, tag=f"sA{n}", name=f"sA{n}")
                   for n in range(NTC)]
        stC16_1 = [ln1_p.tile([1, 512], BF16, tag=f"sC{n}", name=f"sC{n}")
                   for n in range(NTC)]
        with ExitStack() as ph:
            sb = ph.enter_context(tc.tile_pool(name="l1sb", bufs=2))
            ps = ph.enter_context(tc.tile_pool(name="l1ps", bufs=2, space="PSUM"))
            for n in range(NTC):
                ln_stats_n(src1, n, ps, sb, ocol128, FP32R, stA16_1, stC16_1)

        # ---------------- phase 0: adaLN modulation ----------------
        with ExitStack() as ph:
            wm = ph.enter_context(tc.tile_pool(name="p0wm", bufs=2))
            sb = ph.enter_context(tc.tile_pool(name="p0sb", bufs=2))
            rw = ph.enter_context(tc.tile_pool(name="p0rw", bufs=1))
            ps = ph.enter_context(tc.tile_pool(name="p0ps", bufs=3, space="PSUM"))
            bmod_t = rw.tile([1, 6 * H], BF16, tag="bmod", name="bmod")
            nc.scalar.dma_start(bmod_t[:], bmod_s.ap())
            sw8 = rw.tile([128, FK, BPC], BF16, tag="sw8", name="sw8")
            for k in range(FK):
                craw = sb.tile([128, BPC], FP32, tag="craw", name="craw")
                nc.gpsimd.dma_start(craw[:], cT.ap()[k * 128:(k + 1) * 128, :])
                sg = sb.tile([128, BPC], FP32, tag="sg", name="sg")
                if SIM_COMPAT:
                    sg0 = sb.tile([128, BPC], FP32, tag="sg0", name="sg0")
                    nc.scalar.activation(sg0[:], craw[:], AF.Sigmoid, bias=0.0,
                                         scale=1.0)
                    nc.vector.tensor_mul(sg[:], craw[:], sg0[:])
                else:
                    nc.scalar.activation(sg[:], craw[:], AF.Silu, bias=0.0,
                                         scale=1.0)
                nc.vector.tensor_scalar_mul(sw8[:, k, :], sg[:], SSW)
            rows = [[rw.tile([BPC, H], BF16, tag=f"mr{u}{w}", name=f"mr{u}{w}")
                     for w in range(3)] for u in range(2)]
            for ch in range(18):
                j, po = ch // 3, (ch % 3) * 384
                u, w = j // 3, j % 3
                wmod_t = wm.tile([128, FK, 384], BF16, tag="wmod", name="wmod")
                weng = nc.scalar if ch % 2 == 0 else nc.gpsimd
                weng.dma_start(
                    wmod_t[:], wmod8.ap()[:, ch * FK * 384:(ch + 1) * FK * 384])
                pm = ps.tile([BPC, 384], FP32, tag="pm", name="pm")
                for k in range(FK):
                    nc.tensor.matmul(pm[:], sw8[:, k, :], wmod_t[:, k, :],
                                     start=(k == 0), stop=False)
                nc.tensor.matmul(pm[:], ones16[0:1, 0:BPC],
                                 bmod_t[0:1, ch * 384:(ch + 1) * 384],
                                 start=False, stop=True)
                dst = rows[u][w][:, po:po + 384]
                if w == 0:
                    nc.scalar.activation(dst, pm[:], AF.Copy, bias=0.0,
                                         scale=SXN * DMOD)
                elif w == 1:
                    nc.scalar.activation(dst, pm[:], AF.Copy, bias=SXN,
                                         scale=SXN * DMOD)
                else:
                    nc.scalar.activation(dst, pm[:], AF.Copy, bias=0.0,
                                         scale=(CPR if u == 0 else CM2) * DMOD)
                if ch in (8, 17):
                    uu = 0 if ch == 8 else 1
                    for row_w in range(3):
                        idx = uu * 3 + row_w
                        nc.gpsimd.dma_start(rows_d[idx:idx + 1, :],
                                            rows[uu][row_w][:, :])
                    for b in range(BPC):
                        for (row_w, dst) in ((0, sh_col), (1, sc_col),
                                             (2, g_col)):
                            idx = uu * 3 + row_w
                            nc.gpsimd.dma_start(
                                dst[uu][b][:],
                                rows_d[idx:idx + 1, b * H:(b + 1) * H]
                                .rearrange("o (c p) -> (o p) c", p=128))


        # ---------------- phase 1b: LN1 apply -> xn8 ----------------
        with ExitStack() as ph:
            sb = ph.enter_context(tc.tile_pool(name="l1ap", bufs=2))
            pe = ph.enter_context(tc.tile_pool(name="l1pe", bufs=2, space="PSUM"))
            for b in range(BPC):
                ln_apply_b(src1, 0, xn8, sb, pe, stA16_1, stC16_1, b)
        ln1_p.release()
        xc_p.release()

        # ---------------- phase 2: qkv qk-part + stats + norm ----------------
        stq_p = tc.alloc_tile_pool(name="stqp", bufs=1)
        stq_r = stq_p.tile([32, T], BF16, tag="stq_r", name="stq_r")
        stq_mr = stq_p.tile([32, T], BF16, tag="stq_mr", name="stq_mr")
        exr_t = stq_p.tile([32, QKF * 128], BF16, tag="exr", name="exr")
        nc.sync.dma_start(exr_t[:], exr.ap())
        inds_t = stq_p.tile([128, QKF, 64], FP8, tag="inds", name="inds")
        nc.sync.dma_start(inds_t[:], ind8s.ap())
        indq_t = stq_p.tile([128, QKF, 64], FP8, tag="indq", name="indq")
        nc.sync.dma_start(indq_t[:], ind8q.ap())
        qs_p = tc.alloc_tile_pool(name="qsp", bufs=1)
        qs8 = qs_p.tile([128, QKF, T], FP8, tag="qs8", name="qs8")
        with ExitStack() as ph:
            wp = ph.enter_context(tc.tile_pool(name="qkw", bufs=2))
            sb = ph.enter_context(tc.tile_pool(name="qksb", bufs=2))
            ps = ph.enter_context(tc.tile_pool(name="qkps", bufs=2, space="PSUM"))
            st = ph.enter_context(tc.tile_pool(name="qkst", bufs=1, space="PSUM"))
            qstat = [st.tile([64, 512], FP32, tag=f"qst{n}", name=f"qst{n}")
                     for n in range(NTC)]
            sqp = None
            for mb in range(QKF):
                wt = wp.tile([128, FKP, 128], FP8, tag="wt", name="wt")
                nc.sync.dma_start(
                    wt[:], wqk8.ap()[:, mb * FKP * 128:(mb + 1) * FKP * 128])
                if mb % 2 == 0:
                    sqp = sb.tile([128, 2, NTC, 512], FP8, tag="sqp", name="sqp")
                for n2 in range(2):
                    wsl = slice(n2 * 1024, (n2 + 1) * 1024)
                    mm = ps.tile([128, 2, 512], FP32, tag="mm", name="mm")
                    for i in range(2):
                        nsl = slice((2 * n2 + i) * 512, (2 * n2 + i + 1) * 512)
                        for jj in range(FKP // 2):
                            nc.tensor.matmul(mm[:, i, :],
                                             wt[:, 2 * jj:2 * jj + 2, :],
                                             xn8[:, 2 * jj:2 * jj + 2, nsl],
                                             start=(jj == 0),
                                             stop=(jj == FKP // 2 - 1),
                                             perf_mode=DRM)
                    nc.scalar.activation(qs8[:, mb, wsl], mm[:, :, :], AF.Identity,
                                         bias=bqk_t[:, mb:mb + 1], scale=CQ)
                    nc.scalar.activation(sqp[:, mb % 2, 2 * n2:2 * n2 + 2, :],
                                         mm[:, :, :], AF.Square,
                                         bias=bqk_t[:, mb:mb + 1], scale=CQ)
                if mb % 2 == 1:
                    for n in range(NTC):
                        nsl = slice(n * 512, (n + 1) * 512)
                        nc.tensor.matmul(qstat[n][:], inds_t[:, mb - 1:mb + 1, :],
                                         qs8[:, mb - 1:mb + 1, nsl],
                                         start=(mb == 1), stop=False,
                                         perf_mode=DRM, skip_group_check=True)
                        nc.tensor.matmul(qstat[n][:], indq_t[:, mb - 1:mb + 1, :],
                                         sqp[:, :, n, :],
                                         start=False, stop=(mb == QKF - 1),
                                         perf_mode=DRM, skip_group_check=True)
            for n in range(NTC):
                nsl = slice(n * 512, (n + 1) * 512)
                ms64 = sb.tile([64, 512], FP32, tag="ms64", name="ms64")
                nc.scalar.mul(ms64[:], qstat[n][:], 1.0 / HD)
                msq = sb.tile([32, 512], FP32, tag="msqh", name="msqh")
                nc.gpsimd.dma_start(msq[:], ms64[32:64, :])
                m2 = sb.tile([32, 512], FP32, tag="m2h", name="m2h")
                nc.vector.tensor_mul(m2[:], ms64[0:32, :], ms64[0:32, :])
                var = sb.tile([32, 512], FP32, tag="varh", name="varh")
                nc.vector.tensor_sub(var[:], msq[:], m2[:])
                sd = sb.tile([32, 512], FP32, tag="sdh", name="sdh")
                nc.scalar.activation(sd[:], var[:], AF.Sqrt,
                                     bias=eps_t[0:32, 1:2], scale=1.0)
                stqf = sb.tile([32, 512], FP32, tag="stqf", name="stqf")
                nc.vector.reciprocal_approx_fast(stqf[:], sd[:])
                nc.vector.tensor_copy(stq_r[:, nsl], stqf[:])
                nc.vector.tensor_mul(stq_mr[:, nsl], ms64[0:32, :], stqf[:])

        # ---- phase 2.5: qk-norm n2=0 (b0) + v(b0, heads 0-5), interleaved ----
        nc.sync.dma_start(wv_t[:], wv8.ap())
        for b in range(BPC):
            nc.vector.memset(v_sb[b][:], 0.0)
            nc.vector.memset(v_sb[b][:, :, :, 96:97], 1.0)

        def do_v(b, nk, h0, h1, pool):
            tsl = slice((b * 8 + nk) * 128, (b * 8 + nk + 1) * 128)
            nh = (h1 - h0) * HD
            vt = pool.tile([128, 2, 512], FP32, tag="fmm", name="fmm")
            for jj in range(FKP // 2):
                nc.tensor.matmul(vt[:, 0, 0:nh], xn8[:, 2 * jj:2 * jj + 2, tsl],
                                 wv_t[:, 2 * jj:2 * jj + 2, h0 * HD:h1 * HD],
                                 start=(jj == 0), stop=(jj == FKP // 2 - 1),
                                 perf_mode=DRM)
            nc.vector.tensor_scalar_mul(
                v_sb[b][:, nk, h0:h1, 0:HD],
                vt[:, 0, 0:nh].rearrange("p (h d) -> p h d", h=h1 - h0), CV)

        def do_norm_chunk(mb, nsl, pool, sb):
            fsl = slice(mb * 128, (mb + 1) * 128)
            rpz = pool.tile([128, 2, 512], FP32, tag="fmm", name="fmm")
            nc.tensor.matmul(rpz[:, 0, :], exr_t[:, fsl], stq_r[:, nsl],
                             start=True, stop=True)
            nc.tensor.matmul(rpz[:, 1, :], exr_t[:, fsl], stq_mr[:, nsl],
                             start=True, stop=True)
            tqc = sb.tile([128, 512], FP32, tag="tqc", name="tqc", bufs=2)
            nc.vector.tensor_tensor(tqc[:], qs8[:, mb, nsl], rpz[:, 0, :],
                                    op=ALU.mult)
            qn8c = sb.tile([128, 512], FP8, tag="qn8c", name="qn8c", bufs=2)
            nc.vector.tensor_tensor(qn8c[:], tqc[:], rpz[:, 1, :],
                                    op=ALU.subtract)
            nc.sync.dma_start(qk_d8[fsl, nsl], qn8c[:])

        nrm_order = [m for p in zip(range(FK), range(FK, QKF)) for m in p]
        with ExitStack() as ph:
            sb = ph.enter_context(tc.tile_pool(name="nrm", bufs=3))
            pe = ph.enter_context(tc.tile_pool(name="nrmpe", bufs=2, space="PSUM"))
            vix = 0
            for mb in nrm_order:
                for i in range(2):
                    do_norm_chunk(mb, slice(i * 512, (i + 1) * 512), pe, sb)
                if vix < 8:
                    do_v(0, vix, 0, 6, pe)
                    vix += 1

        # ------- merged region: attention + v rest + qk-norm(b1) + proj(b0) ---
        x2_p = None
        x2_sb = None  # allocated at b1 start, after front tiles release
        nc.vector.memset(o_sb[:, FK:FKP, :], 0.0)
        with ExitStack() as ph:
            qp = ph.enter_context(tc.tile_pool(name="aq", bufs=3,
                                               side="right"))
            up = ph.enter_context(tc.tile_pool(name="au", bufs=2,
                                               side="right"))
            ob = ph.enter_context(tc.tile_pool(name="ao", bufs=2,
                                               side="right"))
            wp = ph.enter_context(tc.tile_pool(name="pw", bufs=2,
                                               side="right"))
            sbp = ph.enter_context(tc.tile_pool(name="psb", bufs=3,
                                                side="right"))
            pss = ph.enter_context(tc.tile_pool(name="pss", bufs=2, space="PSUM"))
            pso = ph.enter_context(tc.tile_pool(name="pso", bufs=1, space="PSUM"))
            psz = ph.enter_context(tc.tile_pool(name="psz", bufs=1, space="PSUM"))
            pfl = ph.enter_context(tc.tile_pool(name="pfl", bufs=1, space="PSUM"))

            fillers = deque()

            def pop_filler():
                if fillers:
                    fillers.popleft()()

            for (h0, h1) in VBLK[1:]:
                for nk in range(8):
                    fillers.append(
                        lambda nk=nk, h0=h0, h1=h1: do_v(0, nk, h0, h1, pfl))
            for (h0, h1) in VBLK:
                for nk in range(8):
                    fillers.append(
                        lambda nk=nk, h0=h0, h1=h1: do_v(1, nk, h0, h1, pfl))
            for mb in nrm_order:
                def nrm_unit(mb=mb):
                    for i in range(2):
                        do_norm_chunk(mb, slice(1024 + i * 512,
                                                1024 + (i + 1) * 512), pfl, sbp)
                fillers.append(nrm_unit)

            def attn_scores(b, h, budget):
                c0 = b * N
                r0 = h * HD
                q_t = qp.tile([36, 2, N], FP8, tag="qt", name="qt")
                nc.sync.dma_start(
                    q_t[:], qk_d8[r0:r0 + HD, c0:c0 + N]
                    .rearrange("(two p) c -> p two c", two=2))
                k_t = qp.tile([36, 2, N], FP8, tag="kt", name="kt")
                nc.sync.dma_start(
                    k_t[:], qk_d8[H + r0:H + r0 + HD, c0:c0 + N]
                    .rearrange("(two p) c -> p two c", two=2))
                ut = up.tile([128, 8, N], FP8, tag="ut", name="ut")
                for nk in range(8):
                    ksl = slice(nk * 128, (nk + 1) * 128)
                    sp = pss.tile([128, 2, 512], FP32, tag="sp", name="sp")
                    for q2 in range(2):
                        nc.tensor.matmul(sp[:, q2, :], k_t[:, :, ksl],
                                         q_t[:, :, q2 * 512:(q2 + 1) * 512],
                                         start=True, stop=True, perf_mode=DRM)
                    nc.scalar.activation(ut[:, nk, :], sp[:, :, :], AF.Exp,
                                         bias=eps_t[:, 2:3], scale=CEXP)
                    if nk < budget:
                        pop_filler()
                return ut

            def attn_out(b, h, ut):
                c0 = b * N
                r0 = h * HD
                o8 = ob.tile([HD, N], FP8, tag="o8", name="o8")
                for q2 in range(2):
                    qsl = slice(q2 * 512, (q2 + 1) * 512)
                    avp = pso.tile([VCOL, 512], FP32, tag="avp", name="avp")
                    for jj in range(4):
                        nc.tensor.matmul(avp[:],
                                         v_sb[b][:, 2 * jj:2 * jj + 2, h, :],
                                         ut[:, 2 * jj:2 * jj + 2, qsl],
                                         start=(jj == 0), stop=(jj == 3),
                                         perf_mode=DRM)
                    zrow = ob.tile([1, 512], FP32, tag="zrow", name="zrow")
                    nc.vector.tensor_copy(zrow[:], avp[96:97, :])
                    rzf = ob.tile([1, 512], FP32, tag="rzf", name="rzf")
                    nc.vector.reciprocal_approx_fast(rzf[:], zrow[:])
                    rz = ob.tile([1, 512], BF16, tag="rz", name="rz")
                    nc.vector.tensor_copy(rz[:], rzf[:])
                    osb = ob.tile([HD, 512], FP32, tag="osb", name="osb")
                    nc.vector.tensor_copy(osb[:], avp[0:HD, :])
                    rzp = psz.tile([HD, 512], FP32, tag="rzp", name="rzp")
                    nc.tensor.matmul(rzp[:], ones16[0:1, 0:HD], rz[:],
                                     start=True, stop=True)
                    nc.vector.tensor_tensor(o8[:, qsl], osb[:], rzp[:],
                                            op=ALU.mult)
                c_, p_ = r0 // 128, r0 % 128
                n0 = min(HD, 128 - p_)
                nc.gpsimd.dma_start(o_sb[p_:p_ + n0, c_, c0:c0 + N], o8[0:n0, :])
                if n0 < HD:
                    nc.gpsimd.dma_start(o_sb[0:HD - n0, c_ + 1, c0:c0 + N],
                                        o8[n0:HD, :])

            def proj_unit(n, mb):
                def emit():
                    b = n // (NTC // BPC)
                    nsl = slice(n * 512, (n + 1) * 512)
                    msl = slice(mb * 128, (mb + 1) * 128)
                    wt = wp.tile([128, FKP, 128], FP8, tag="wpt", name="wpt")
                    nc.gpsimd.dma_start(
                        wt[:], wpro8.ap()[:, mb * FKP * 128:(mb + 1) * FKP * 128])
                    mm = pfl.tile([128, 2, 512], FP32, tag="fmm", name="fmm")
                    for jj in range(FKP // 2):
                        nc.tensor.matmul(mm[:, 0, :],
                                         wt[:, 2 * jj:2 * jj + 2, :],
                                         o_sb[:, 2 * jj:2 * jj + 2, nsl],
                                         start=(jj == 0),
                                         stop=(jj == FKP // 2 - 1),
                                         perf_mode=DRM)
                    xr = sbp.tile([128, 512], FP32R, tag="xr", name="xr", bufs=2)
                    nc.sync.dma_start(xr[:], xT.ap()[msl, nsl])
                    nc.vector.scalar_tensor_tensor(
                        x2_sb[:, mb, nsl], mm[:, 0, :],
                        g_col[0][b][:, mb:mb + 1], xr[:],
                        op0=ALU.mult, op1=ALU.add)
                return emit

            prev = None
            for b in range(BPC):
                for h in range(NH):
                    budget = min(8, -(-len(fillers) // max(1, NH - h)))
                    ut = attn_scores(b, h, budget)
                    if prev is not None:
                        attn_out(*prev)
                    if b == 1 and h == 0:
                        v_ps[0].release()
                        x2_p = tc.alloc_tile_pool(name="x2p", bufs=1)
                        x2_sb = x2_p.tile([128, FK, T], BF16, tag="x2",
                                          name="x2")
                        for n in (0, 1):
                            for mb in range(FK):
                                fillers.append(proj_unit(n, mb))
                    prev = (b, h, ut)
                if b == 0:
                    while fillers:
                        fillers.popleft()()
                    qs_p.release()
                    stq_p.release()
                    xn_p.release()
                    vw_p.release()
            attn_out(*prev)
            while fillers:
                fillers.popleft()()

        # ------- phase 6: proj(b1) + LN2 -> xn2; mlp1 -> h (fused) -------
        with ExitStack() as ph:
            sb = ph.enter_context(tc.tile_pool(name="l2sb", bufs=2))
            ro = ph.enter_context(tc.tile_pool(name="l2ro", bufs=1))
            stA16_2 = [ro.tile([1, 512], BF16, tag=f"sA{n}", name=f"sA{n}")
                       for n in range(NTC)]
            stC16_2 = [ro.tile([1, 512], BF16, tag=f"sC{n}", name=f"sC{n}")
                       for n in range(NTC)]
            src2 = lambda k: x2_sb[:, k, :]
            st_ps = ExitStack()
            ps = st_ps.enter_context(tc.tile_pool(name="l2ps", bufs=2,
                                                  space="PSUM"))
            pj = st_ps.enter_context(tc.tile_pool(name="l2pj", bufs=2,
                                                  space="PSUM"))
            pwp = st_ps.enter_context(tc.tile_pool(name="l2pw", bufs=2))
            pxr = st_ps.enter_context(tc.tile_pool(name="l2xr", bufs=2))

            def proj_mb(n, mb):
                b = n // (NTC // BPC)
                nsl = slice(n * 512, (n + 1) * 512)
                msl = slice(mb * 128, (mb + 1) * 128)
                wt = pwp.tile([128, FKP, 128], FP8, tag="wpt", name="wpt")
                nc.gpsimd.dma_start(
                    wt[:], wpro8.ap()[:, mb * FKP * 128:(mb + 1) * FKP * 128])
                mm = pj.tile([128, 512], FP32, tag="pjm", name="pjm")
                for jj in range(FKP // 2):
                    nc.tensor.matmul(mm[:], wt[:, 2 * jj:2 * jj + 2, :],
                                     o_sb[:, 2 * jj:2 * jj + 2, nsl],
                                     start=(jj == 0), stop=(jj == FKP // 2 - 1),
                                     perf_mode=DRM)
                xr = pxr.tile([128, 512], FP32R, tag="xr", name="xr")
                nc.sync.dma_start(xr[:], xT.ap()[msl, nsl])
                nc.vector.scalar_tensor_tensor(
                    x2_sb[:, mb, nsl], mm[:], g_col[0][b][:, mb:mb + 1], xr[:],
                    op0=ALU.mult, op1=ALU.add)

            for mb in range(FK):
                proj_mb(2, mb)
            ln_stats_n(src2, 0, ps, sb, ocol16, BF16, stA16_2, stC16_2)
            for mb in range(FK):
                proj_mb(3, mb)
            ln_stats_n(src2, 1, ps, sb, ocol16, BF16, stA16_2, stC16_2)
            ln_stats_n(src2, 2, ps, sb, ocol16, BF16, stA16_2, stC16_2)
            ln_stats_n(src2, 3, ps, sb, ocol16, BF16, stA16_2, stC16_2)
            st_ps.close()

            xn2_p = tc.alloc_tile_pool(name="xn2p", bufs=1, side="right")
            xn2 = xn2_p.tile([128, FKP, T], FP8, tag="xn2", name="xn2")
            nc.vector.memset(xn2[:, FK:FKP, :], 0.0)
            h_p = tc.alloc_tile_pool(name="hp", bufs=1, side="right")
            h_sb = h_p.tile([128, MK, T], FP8, tag="h_sb", name="h_sb")
            pe = ph.enter_context(tc.tile_pool(name="l2pe", bufs=1, space="PSUM"))
            wp = ph.enter_context(tc.tile_pool(name="m1w", bufs=2))
            msb = ph.enter_context(tc.tile_pool(name="m1sb", bufs=3,
                                                side="right"))
            mps = ph.enter_context(tc.tile_pool(name="m1ps", bufs=2, space="PSUM"))
            for n2 in range(BPC):
                ln_apply_b(src2, 1, xn2, sb, pe, stA16_2, stC16_2, n2)
                wsl = slice(n2 * 1024, (n2 + 1) * 1024)
                for mb in range(MK):
                    wt = wp.tile([128, FKP, 128], FP8, tag="wt", name="wt")
                    nc.sync.dma_start(
                        wt[:], w18.ap()[:, mb * FKP * 128:(mb + 1) * FKP * 128])
                    mm = mps.tile([128, 2, 512], FP32, tag="mm", name="mm")
                    for i in range(2):
                        nsl = slice((2 * n2 + i) * 512, (2 * n2 + i + 1) * 512)
                        for jj in range(FKP // 2):
                            nc.tensor.matmul(mm[:, i, :],
                                             wt[:, 2 * jj:2 * jj + 2, :],
                                             xn2[:, 2 * jj:2 * jj + 2, nsl],
                                             start=(jj == 0),
                                             stop=(jj == FKP // 2 - 1),
                                             perf_mode=DRM)
                    if SIM_COMPAT:
                        z = msb.tile([128, 1024], FP32, tag="gz", name="gz",
                                     bufs=1)
                        nc.scalar.activation(z[:], mm[:, :, :], AF.Identity,
                                             bias=0.0, scale=CG1)
                        t1 = msb.tile([128, 1024], FP32, tag="gt1", name="gt1",
                                      bufs=1)
                        nc.vector.tensor_mul(t1[:], z[:], z[:])
                        nc.vector.tensor_mul(t1[:], t1[:], z[:])
                        nc.vector.scalar_tensor_tensor(t1[:], t1[:], 0.044715,
                                                       z[:], op0=ALU.mult,
                                                       op1=ALU.add)
                        t4 = msb.tile([128, 1024], FP32, tag="gt4", name="gt4",
                                      bufs=1)
                        nc.scalar.activation(t4[:], t1[:], AF.Tanh, bias=0.0,
                                             scale=0.7978845608028654)
                        nc.vector.scalar_tensor_tensor(t4[:], t4[:], 1.0, z[:],
                                                       op0=ALU.add, op1=ALU.mult)
                        nc.vector.tensor_scalar_mul(h_sb[:, mb, wsl], t4[:], 0.5)
                    else:
                        nc.scalar.activation(h_sb[:, mb, wsl], mm[:, :, :],
                                             AF.Gelu_apprx_tanh, bias=0.0,
                                             scale=CG1)

        # ---------------- phase 7: mlp2 + gated residual -> out ----------------
        with ExitStack() as ph:
            wp = ph.enter_context(tc.tile_pool(name="m2w", bufs=2))
            sb = ph.enter_context(tc.tile_pool(name="m2sb", bufs=3))
            ps = ph.enter_context(tc.tile_pool(name="m2ps", bufs=2, space="PSUM"))
            for mb in range(FK):
                wt = wp.tile([128, MK, 128], FP8, tag="wt", name="wt")
                nc.sync.dma_start(
                    wt[:], w28.ap()[:, mb * MK * 128:(mb + 1) * MK * 128])
                msl = slice(mb * 128, (mb + 1) * 128)
                for b in range(BPC):
                    wsl = slice(b * N, (b + 1) * N)
                    mm = ps.tile([128, 2, 512], FP32, tag="mm", name="mm")
                    for i in range(2):
                        nsl = slice((2 * b + i) * 512, (2 * b + i + 1) * 512)
                        for jj in range(MK // 2):
                            nc.tensor.matmul(mm[:, i, :],
                                             wt[:, 2 * jj:2 * jj + 2, :],
                                             h_sb[:, 2 * jj:2 * jj + 2, nsl],
                                             start=(jj == 0),
                                             stop=(jj == MK // 2 - 1),
                                             perf_mode=DRM)
                    os_ = sb.tile([128, 1024], FP32, tag="os", name="os")
                    nc.vector.scalar_tensor_tensor(
                        os_[:], mm[:, :, :], g_col[1][b][:, mb:mb + 1],
                        x2_sb[:, mb, wsl], op0=ALU.mult, op1=ALU.add)
                    nc.sync.dma_start(out.ap()[msl, wsl], os_[:])
        h_p.release()
        xn2_p.release()
        x2_p.release()
        v_ps[1].release()
        o_p.release()

    nc.finalize()
    return nc


def _f8(x):
    return np.asarray(x, np.float32).astype(E4)


def _pack_dr(w, scale, kp, fblk=128):
    """[Kin, Mout] fp32 -> [128, (Mout//fblk)*kp*fblk] fp8, DR-stationary order
    [mb][plane][f]: pack[p, (mb*kp+pl)*fblk + f] = scale*w[pl*128+p, mb*fblk+f]."""
    kin, mout = w.shape
    wp = np.zeros((kp * 128, mout), np.float32)
    wp[:kin] = np.asarray(w, np.float32) * scale
    a = wp.reshape(kp, 128, mout // fblk, fblk)
    return _f8(np.ascontiguousarray(a.transpose(1, 2, 0, 3).reshape(128, -1)))


def _host_inputs(x, c, w_mod, b_mod, w_qkv, b_qkv, g_q, g_k, w_proj, b_proj,
                 w1, b1, w2, b2):
    f32 = np.float32
    w_qkv = np.asarray(w_qkv, f32)

    # v moving layout: [p, pl*H + col]
    wv = np.zeros((FKP * 128, H), f32)
    wv[:H] = w_qkv[:, 2 * H:] * SWV
    wv8m = _f8(wv.reshape(FKP, 128, H).transpose(1, 0, 2).reshape(128, -1))

    exr = np.zeros((32, QKF * 128), f32)
    gq = np.asarray(g_q, f32)
    gk = np.asarray(g_k, f32)
    for gf in range(2 * H):
        if gf < H:
            s, g = gf // HD, gq[gf % HD]
        else:
            s, g = 16 + (gf - H) // HD, gk[(gf - H) % HD]
        exr[s, gf] = SQN * g

    inds = np.zeros((128, QKF * 64), f32)
    indq = np.zeros((128, QKF * 64), f32)
    for mb in range(QKF):
        for f in range(128):
            gf = mb * 128 + f
            if gf < H:
                hh, base = gf // HD, 0
            else:
                hh, base = (gf - H) // HD, 16
            inds[f, mb * 64 + base + hh] = 1.0
            indq[f, mb * 64 + 32 + base + hh] = 1.0

    epsc = np.zeros((128, 4), f32)
    epsc[:, 0] = EPS
    epsc[:, 1] = EPS * SQ * SQ
    epsc[:, 2] = EXPB

    shared = {
        "wmod8": np.ascontiguousarray(
            np.asarray(w_mod, f32).reshape(FK, 128, 18, 384)
            .transpose(1, 2, 0, 3).reshape(128, -1)).astype(ml_dtypes.bfloat16),
        "bmod_s": (np.asarray(b_mod, f32).reshape(1, 6 * H)
                   * (SMOD * SSW)).astype(ml_dtypes.bfloat16),
        "wqk8": _pack_dr(np.ascontiguousarray(w_qkv[:, :2 * H]), SWQK, FKP),
        "wv8": wv8m,
        "bqk": np.ascontiguousarray(
            np.asarray(b_qkv, f32)[:2 * H].reshape(QKF, 128).T) * SQ,
        "exr": exr.astype(ml_dtypes.bfloat16),
        "ind8s": _f8(inds),
        "ind8q": _f8(indq),
        "wpro8": _pack_dr(np.asarray(w_proj, f32), SWPRO, FKP),
        "w18": _pack_dr(np.asarray(w1, f32), SW1, FKP),
        "w28": _pack_dr(np.asarray(w2, f32), SW2, MK),
        "onesr": np.ones((1, 512), f32),
        "onesb": np.ones((1, 512), f32).astype(ml_dtypes.bfloat16),
        "onesc": np.ones((128, 1), f32),
        "epsc": epsc,
    }

    in_maps = []
    for core in range(NCORES):
        xs = np.asarray(x[core * BPC:(core + 1) * BPC], f32)   # [2, N, H]
        m = dict(shared)
        m["xT"] = np.ascontiguousarray(xs.reshape(T, H).T)
        m["cT"] = np.ascontiguousarray(
            np.asarray(c[core * BPC:(core + 1) * BPC], f32).T)
        in_maps.append(m)
    return in_maps


def kernel(**inputs):
    if "nc" not in _CACHE:
        _CACHE["nc"] = _build_program()
    nc = _CACHE["nc"]
    in_maps = _host_inputs(**inputs)
    res = run_bass_kernel_spmd(nc, in_maps, core_ids=list(range(NCORES)))
    outs = [res.results[core]["out"].T.reshape(BPC, N, H) for core in range(NCORES)]
    return np.concatenate(outs, axis=0).astype(np.float32)


# revision 14
# speedup vs baseline: 1.1851x; 1.1851x over previous
"""DiT block kernel for 8 Trainium2 NeuronCores.

Data-parallel over batch (2 per core). All big GEMMs run fp8e4m3 with
DoubleRow perf mode. Power-of-2 scale factors keep fp8 tensors in the e4m3
sweet spot; descales fold into existing ACT/DVE ops.

v2 restructure: the softmax exp stream on the ACT engine paces attention
(~9.2us/head) while the PE only has ~6.2us/head of score/av matmuls; the
v1 kernel let the PE idle in ~3us gaps which re-throttled the HAM clock
gate (K=4/8, 1.2GHz) for the whole attention region. Here the v GEMM, the
b1 half of qk-norm, and proj(b0) are emitted as filler units between score
blocks so the PE stays saturated (and warm) through attention. PSUM->SBUF
copies moved off ACT onto DVE so ACT runs pure exp.
"""

import sys

sys.path.insert(0, "/opt/trn_rl_repo")

from collections import deque
from contextlib import ExitStack

import ml_dtypes
import numpy as np

import concourse.bacc as bacc
import concourse.tile as tile
from concourse import mybir
from concourse.bass_utils import run_bass_kernel_spmd

FP32 = mybir.dt.float32
FP32R = mybir.dt.float32r
FP8 = mybir.dt.float8e4
BF16 = mybir.dt.bfloat16
AF = mybir.ActivationFunctionType
ALU = mybir.AluOpType
DRM = mybir.MatmulPerfMode.DoubleRow
E4 = ml_dtypes.float8_e4m3

B, N, H = 16, 1024, 1152
NH, HD = 16, 72
MLP = H * 4
NCORES = 8
BPC = B // NCORES            # 2
T = BPC * N                  # 2048
NTC = T // 512               # 4
FK = H // 128                # 9
FKP = 10                     # padded K planes for H-contractions
QKF = (2 * H) // 128         # 18
MK = MLP // 128              # 36
EPS = 1e-6
ISQ = float(HD) ** -0.5
EXPB = -2.5
VCOL = 97                    # z ones-row at partition 96 (32-aligned for DVE)

# power-of-2 fp8 scales
SXN = 16.0
SWQK = 4096.0
SQ = 2.0  # keep (SQ*q)^2 well under the e4m3 448 max
SQN = 16.0
SWV = 4096.0
SV = 16.0
SO = 16.0
SWPRO = 2048.0
SW1 = 4096.0
SW2 = 4096.0
SMOD = 1.0   # mod path runs bf16: no range scaling needed
SSW = 1.0
DMOD = 1.0 / (SMOD * SSW)
CQ = SQ / (SWQK * SXN)
CV = SV / (SWV * SXN)
CPR = 1.0 / (SWPRO * SO)
CM2 = 1.0 / SW2
CEXP = ISQ / (SQN * SQN)
CG1 = 1.0 / (SW1 * SXN)
assert SO == SV  # rzp expansion row uses plain ones

_CACHE = {}
SIM_COMPAT = False  # decompose Silu/Gelu for CoreSim (no LUTs there)
VBLK = ((0, 6), (6, 12), (12, 16))  # head-group blocks for the v matmul


def _build_program():
    nc = bacc.Bacc("TRN2", target_bir_lowering=False, debug=False)

    xT = nc.dram_tensor("xT", [H, T], FP32R, kind="ExternalInput")
    cT = nc.dram_tensor("cT", [H, BPC], FP32, kind="ExternalInput")
    wmod8 = nc.dram_tensor("wmod8", [128, 18 * FK * 384], BF16, kind="ExternalInput")
    bmod_s = nc.dram_tensor("bmod_s", [1, 6 * H], BF16, kind="ExternalInput")
    wqk8 = nc.dram_tensor("wqk8", [128, QKF * FKP * 128], FP8, kind="ExternalInput")
    wv8 = nc.dram_tensor("wv8", [128, FKP * H], FP8, kind="ExternalInput")
    bqk = nc.dram_tensor("bqk", [128, QKF], FP32, kind="ExternalInput")
    exr = nc.dram_tensor("exr", [32, QKF * 128], BF16, kind="ExternalInput")
    ind8s = nc.dram_tensor("ind8s", [128, QKF * 64], FP8, kind="ExternalInput")
    ind8q = nc.dram_tensor("ind8q", [128, QKF * 64], FP8, kind="ExternalInput")
    wpro8 = nc.dram_tensor("wpro8", [128, FK * FKP * 128], FP8, kind="ExternalInput")
    w18 = nc.dram_tensor("w18", [128, MK * FKP * 128], FP8, kind="ExternalInput")
    w28 = nc.dram_tensor("w28", [128, FK * MK * 128], FP8, kind="ExternalInput")
    onesr = nc.dram_tensor("onesr", [1, 512], FP32R, kind="ExternalInput")
    onesb = nc.dram_tensor("onesb", [1, 512], BF16, kind="ExternalInput")
    onesc = nc.dram_tensor("onesc", [128, 1], FP32R, kind="ExternalInput")
    epsc = nc.dram_tensor("epsc", [128, 4], FP32, kind="ExternalInput")
    out = nc.dram_tensor("out", [H, T], FP32, kind="ExternalOutput")

    with nc.allow_low_precision(
        reason="fp8 matmuls; adaLN gates damp branch error"
    ), tile.TileContext(nc) as tc, ExitStack() as top:
        dram = top.enter_context(tc.tile_pool(name="dram", bufs=1, space="DRAM"))
        qk_d8 = dram.tile([2 * H, T], FP8, tag="qk_d8", name="qk_d8")
        rows_d = dram.tile([6, BPC * H], BF16, tag="rows_d", name="rows_d")

        cst = top.enter_context(tc.tile_pool(name="cst", bufs=1))
        ones512 = cst.tile([1, 512], FP32R, tag="o512", name="o512")
        nc.sync.dma_start(ones512[:], onesr.ap())
        ones16 = cst.tile([1, 512], BF16, tag="o16", name="o16")
        nc.sync.dma_start(ones16[:], onesb.ap())
        ocol128 = cst.tile([128, 1], FP32R, tag="oc128", name="oc128")
        nc.sync.dma_start(ocol128[:], onesc.ap())
        ocol16 = cst.tile([128, 1], BF16, tag="oc16", name="oc16")
        nc.gpsimd.memset(ocol16[:], 1.0)
        eps_t = cst.tile([128, 4], FP32, tag="epsc", name="epsc")
        nc.sync.dma_start(eps_t[:], epsc.ap())
        bqk_t = cst.tile([128, QKF], FP32, tag="bqk", name="bqk")
        nc.sync.dma_start(bqk_t[:], bqk.ap())

        mod_p = top.enter_context(tc.tile_pool(name="modp", bufs=1))
        sc_col = [[mod_p.tile([128, FK], FP32, tag=f"scc{u}{b}", name=f"scc{u}{b}")
                   for b in range(BPC)] for u in range(2)]
        sh_col = [[mod_p.tile([128, FK], FP32, tag=f"shc{u}{b}", name=f"shc{u}{b}")
                   for b in range(BPC)] for u in range(2)]
        g_col = [[mod_p.tile([128, FK], FP32, tag=f"gc{u}{b}", name=f"gc{u}{b}")
                  for b in range(BPC)] for u in range(2)]

        # ---------------- LayerNorm helpers ----------------
        def ln_stats_n(src_of, n, ps, sb, ocol, sqdt, stA16, stC16):
            nsl = slice(n * 512, (n + 1) * 512)
            ln_s = ps.tile([1, 512], FP32, tag="lns", name="lns")
            ln_q = ps.tile([1, 512], FP32, tag="lnq", name="lnq")
            for k in range(FK):
                sq = sb.tile([128, 512], sqdt, tag="sq", name="sq", bufs=2)
                nc.vector.tensor_mul(sq[:], src_of(k)[:, nsl], src_of(k)[:, nsl])
                nc.tensor.matmul(ln_s[:], ocol[:], src_of(k)[:, nsl],
                                 start=(k == 0), stop=(k == FK - 1))
                nc.tensor.matmul(ln_q[:], ocol[:], sq[:],
                                 start=(k == 0), stop=(k == FK - 1))
            ms = sb.tile([1, 512], FP32, tag="ms", name="ms", bufs=1)
            nc.scalar.mul(ms[:], ln_s[:], 1.0 / H)
            msq = sb.tile([1, 512], FP32, tag="msq", name="msq", bufs=1)
            nc.scalar.mul(msq[:], ln_q[:], 1.0 / H)
            m2 = sb.tile([1, 512], FP32, tag="sc4", name="m2", bufs=2)
            nc.vector.tensor_mul(m2[:], ms[:], ms[:])
            var = sb.tile([1, 512], FP32, tag="sc4", name="var", bufs=2)
            nc.vector.tensor_sub(var[:], msq[:], m2[:])
            sd = sb.tile([1, 512], FP32, tag="sc4", name="sd", bufs=2)
            nc.scalar.activation(sd[:], var[:], AF.Sqrt, bias=eps_t[0:1, 0:1],
                                 scale=1.0)
            stA_ = sb.tile([1, 512], FP32, tag="sc4", name="stAf", bufs=2)
            nc.vector.reciprocal_approx_fast(stA_[:], sd[:])
            nc.vector.tensor_copy(stA16[n][:], stA_[:])
            nc.vector.scalar_tensor_tensor(stC16[n][:], ms[:], -1.0, stA_[:],
                                           op0=ALU.mult, op1=ALU.mult)

        def ln_apply_b(src_of, u, dst8, sb, pe, stA16, stC16, b):
                wsl = slice(b * N, (b + 1) * N)
                bcA = pe.tile([128, 2, 512], FP32, tag="bcA", name="bcA")
                bcC = pe.tile([128, 2, 512], FP32, tag="bcC", name="bcC")
                for i in range(2):
                    nc.tensor.matmul(bcA[:, i, :], ones16[0:1, 0:128],
                                     stA16[2 * b + i][:], start=True, stop=True)
                    nc.tensor.matmul(bcC[:, i, :], ones16[0:1, 0:128],
                                     stC16[2 * b + i][:], start=True, stop=True)
                for k in range(FK):
                    y = sb.tile([128, 1024], BF16, tag="y", name="y", bufs=2)
                    nc.vector.tensor_tensor(y[:], src_of(k)[:, wsl], bcA[:, :, :],
                                            op=ALU.mult)
                    y2 = sb.tile([128, 1024], BF16, tag="y2", name="y2", bufs=2)
                    nc.vector.tensor_tensor(y2[:], y[:], bcC[:, :, :], op=ALU.add)
                    nc.gpsimd.tensor_scalar(dst8[:, k, wsl], y2[:],
                                            sc_col[u][b][:, k:k + 1],
                                            sh_col[u][b][:, k:k + 1],
                                            op0=ALU.mult, op1=ALU.add)

        # ------- persistent big tiles (left-stack order = reverse release) ---
        o_p = tc.alloc_tile_pool(name="op", bufs=1)
        o_sb = o_p.tile([128, FKP, T], FP8, tag="o_sb", name="o_sb")
        v_ps = [None, None]
        v_sb = [None, None]
        for b in (1, 0):
            v_ps[b] = tc.alloc_tile_pool(name=f"vp{b}", bufs=1)
            v_sb[b] = v_ps[b].tile([128, 8, NH, VCOL], FP8, tag=f"vsb{b}",
                                   name=f"vsb{b}")
        vw_p = tc.alloc_tile_pool(name="vwp", bufs=1)
        wv_t = vw_p.tile([128, FKP, H], FP8, tag="wv", name="wv")
        xn_p = tc.alloc_tile_pool(name="xnp", bufs=1)
        xn8 = xn_p.tile([128, FKP, T], FP8, tag="xn8", name="xn8")
        nc.vector.memset(xn8[:, FK:FKP, :], 0.0)

        # ---------------- phase 1a: x loads + LN1 stats (PE busy early) ----
        xc_p = tc.alloc_tile_pool(name="xcp", bufs=1, side="right")
        xc = xc_p.tile([128, FK, T], FP32R, tag="xc", name="xc")
        for k in range(FK):
            nc.sync.dma_start(xc[:, k, :], xT.ap()[k * 128:(k + 1) * 128, :])
        src1 = lambda k: xc[:, k, :]
        ln1_p = tc.alloc_tile_pool(name="ln1p", bufs=1, side="right")
        stA16_1 = [ln1_p.tile([1, 512], BF16, tag=f"sA{n}", name=f"sA{n}")
                   for n in range(NTC)]
        stC16_1 = [ln1_p.tile([1, 512], BF16, tag=f"sC{n}", name=f"sC{n}")
                   for n in range(NTC)]
        with ExitStack() as ph:
            sb = ph.enter_context(tc.tile_pool(name="l1sb", bufs=2))
            ps = ph.enter_context(tc.tile_pool(name="l1ps", bufs=2, space="PSUM"))
            for n in range(NTC):
                ln_stats_n(src1, n, ps, sb, ocol128, FP32R, stA16_1, stC16_1)

        # ---------------- phase 0: adaLN modulation ----------------
        with ExitStack() as ph:
            wm = ph.enter_context(tc.tile_pool(name="p0wm", bufs=2))
            sb = ph.enter_context(tc.tile_pool(name="p0sb", bufs=2))
            rw = ph.enter_context(tc.tile_pool(name="p0rw", bufs=1))
            ps = ph.enter_context(tc.tile_pool(name="p0ps", bufs=3, space="PSUM"))
            bmod_t = rw.tile([1, 6 * H], BF16, tag="bmod", name="bmod")
            nc.scalar.dma_start(bmod_t[:], bmod_s.ap())
            sw8 = rw.tile([128, FK, BPC], BF16, tag="sw8", name="sw8")
            for k in range(FK):
                craw = sb.tile([128, BPC], FP32, tag="craw", name="craw")
                nc.gpsimd.dma_start(craw[:], cT.ap()[k * 128:(k + 1) * 128, :])
                sg = sb.tile([128, BPC], FP32, tag="sg", name="sg")
                if SIM_COMPAT:
                    sg0 = sb.tile([128, BPC], FP32, tag="sg0", name="sg0")
                    nc.scalar.activation(sg0[:], craw[:], AF.Sigmoid, bias=0.0,
                                         scale=1.0)
                    nc.vector.tensor_mul(sg[:], craw[:], sg0[:])
                else:
                    nc.scalar.activation(sg[:], craw[:], AF.Silu, bias=0.0,
                                         scale=1.0)
                nc.vector.tensor_scalar_mul(sw8[:, k, :], sg[:], SSW)
            rows = [[rw.tile([BPC, H], BF16, tag=f"mr{u}{w}", name=f"mr{u}{w}")
                     for w in range(3)] for u in range(2)]
            for ch in range(18):
                j, po = ch // 3, (ch % 3) * 384
                u, w = j // 3, j % 3
                wmod_t = wm.tile([128, FK, 384], BF16, tag="wmod", name="wmod")
                weng = nc.scalar if ch % 2 == 0 else nc.gpsimd
                weng.dma_start(
                    wmod_t[:], wmod8.ap()[:, ch * FK * 384:(ch + 1) * FK * 384])
                pm = ps.tile([BPC, 384], FP32, tag="pm", name="pm")
                for k in range(FK):
                    nc.tensor.matmul(pm[:], sw8[:, k, :], wmod_t[:, k, :],
                                     start=(k == 0), stop=False)
                nc.tensor.matmul(pm[:], ones16[0:1, 0:BPC],
                                 bmod_t[0:1, ch * 384:(ch + 1) * 384],
                                 start=False, stop=True)
                dst = rows[u][w][:, po:po + 384]
                if w == 0:
                    nc.scalar.activation(dst, pm[:], AF.Copy, bias=0.0,
                                         scale=SXN * DMOD)
                elif w == 1:
                    nc.scalar.activation(dst, pm[:], AF.Copy, bias=SXN,
                                         scale=SXN * DMOD)
                else:
                    nc.scalar.activation(dst, pm[:], AF.Copy, bias=0.0,
                                         scale=(CPR if u == 0 else CM2) * DMOD)
                if ch in (8, 17):
                    uu = 0 if ch == 8 else 1
                    for row_w in range(3):
                        idx = uu * 3 + row_w
                        nc.gpsimd.dma_start(rows_d[idx:idx + 1, :],
                                            rows[uu][row_w][:, :])
                    for b in range(BPC):
                        for (row_w, dst) in ((0, sh_col), (1, sc_col),
                                             (2, g_col)):
                            idx = uu * 3 + row_w
                            nc.gpsimd.dma_start(
                                dst[uu][b][:],
                                rows_d[idx:idx + 1, b * H:(b + 1) * H]
                                .rearrange("o (c p) -> (o p) c", p=128))


        # ---------------- phase 1b: LN1 apply -> xn8 ----------------
        with ExitStack() as ph:
            sb = ph.enter_context(tc.tile_pool(name="l1ap", bufs=2))
            pe = ph.enter_context(tc.tile_pool(name="l1pe", bufs=2, space="PSUM"))
            for b in range(BPC):
                ln_apply_b(src1, 0, xn8, sb, pe, stA16_1, stC16_1, b)
        ln1_p.release()
        xc_p.release()

        # ---------------- phase 2: qkv qk-part + stats + norm ----------------
        stq_p = tc.alloc_tile_pool(name="stqp", bufs=1)
        stq_r = stq_p.tile([32, T], BF16, tag="stq_r", name="stq_r")
        stq_mr = stq_p.tile([32, T], BF16, tag="stq_mr", name="stq_mr")
        exr_t = stq_p.tile([32, QKF * 128], BF16, tag="exr", name="exr")
        nc.sync.dma_start(exr_t[:], exr.ap())
        inds_t = stq_p.tile([128, QKF, 64], FP8, tag="inds", name="inds")
        nc.sync.dma_start(inds_t[:], ind8s.ap())
        indq_t = stq_p.tile([128, QKF, 64], FP8, tag="indq", name="indq")
        nc.sync.dma_start(indq_t[:], ind8q.ap())
        qs_p = tc.alloc_tile_pool(name="qsp", bufs=1)
        qs8 = qs_p.tile([128, QKF, T], FP8, tag="qs8", name="qs8")
        with ExitStack() as ph:
            wp = ph.enter_context(tc.tile_pool(name="qkw", bufs=2))
            sb = ph.enter_context(tc.tile_pool(name="qksb", bufs=2))
            ps = ph.enter_context(tc.tile_pool(name="qkps", bufs=2, space="PSUM"))
            st = ph.enter_context(tc.tile_pool(name="qkst", bufs=1, space="PSUM"))
            qstat = [st.tile([64, 512], FP32, tag=f"qst{n}", name=f"qst{n}")
                     for n in range(NTC)]
            sqp = None
            for mb in range(QKF):
                wt = wp.tile([128, FKP, 128], FP8, tag="wt", name="wt")
                nc.sync.dma_start(
                    wt[:], wqk8.ap()[:, mb * FKP * 128:(mb + 1) * FKP * 128])
                if mb % 2 == 0:
                    sqp = sb.tile([128, 2, NTC, 512], FP8, tag="sqp", name="sqp")
                for n2 in range(2):
                    wsl = slice(n2 * 1024, (n2 + 1) * 1024)
                    mm = ps.tile([128, 2, 512], FP32, tag="mm", name="mm")
                    for i in range(2):
                        nsl = slice((2 * n2 + i) * 512, (2 * n2 + i + 1) * 512)
                        for jj in range(FKP // 2):
                            nc.tensor.matmul(mm[:, i, :],
                                             wt[:, 2 * jj:2 * jj + 2, :],
                                             xn8[:, 2 * jj:2 * jj + 2, nsl],
                                             start=(jj == 0),
                                             stop=(jj == FKP // 2 - 1),
                                             perf_mode=DRM)
                    nc.scalar.activation(qs8[:, mb, wsl], mm[:, :, :], AF.Identity,
                                         bias=bqk_t[:, mb:mb + 1], scale=CQ)
                    nc.scalar.activation(sqp[:, mb % 2, 2 * n2:2 * n2 + 2, :],
                                         mm[:, :, :], AF.Square,
                                         bias=bqk_t[:, mb:mb + 1], scale=CQ)
                if mb % 2 == 1:
                    for n in range(NTC):
                        nsl = slice(n * 512, (n + 1) * 512)
                        nc.tensor.matmul(qstat[n][:], inds_t[:, mb - 1:mb + 1, :],
                                         qs8[:, mb - 1:mb + 1, nsl],
                                         start=(mb == 1), stop=False,
                                         perf_mode=DRM, skip_group_check=True)
                        nc.tensor.matmul(qstat[n][:], indq_t[:, mb - 1:mb + 1, :],
                                         sqp[:, :, n, :],
                                         start=False, stop=(mb == QKF - 1),
                                         perf_mode=DRM, skip_group_check=True)
            for n in range(NTC):
                nsl = slice(n * 512, (n + 1) * 512)
                ms64 = sb.tile([64, 512], FP32, tag="ms64", name="ms64")
                nc.scalar.mul(ms64[:], qstat[n][:], 1.0 / HD)
                msq = sb.tile([32, 512], FP32, tag="msqh", name="msqh")
                nc.gpsimd.dma_start(msq[:], ms64[32:64, :])
                m2 = sb.tile([32, 512], FP32, tag="m2h", name="m2h")
                nc.vector.tensor_mul(m2[:], ms64[0:32, :], ms64[0:32, :])
                var = sb.tile([32, 512], FP32, tag="varh", name="varh")
                nc.vector.tensor_sub(var[:], msq[:], m2[:])
                sd = sb.tile([32, 512], FP32, tag="sdh", name="sdh")
                nc.scalar.activation(sd[:], var[:], AF.Sqrt,
                                     bias=eps_t[0:32, 1:2], scale=1.0)
                stqf = sb.tile([32, 512], FP32, tag="stqf", name="stqf")
                nc.vector.reciprocal_approx_fast(stqf[:], sd[:])
                nc.vector.tensor_copy(stq_r[:, nsl], stqf[:])
                nc.vector.tensor_mul(stq_mr[:, nsl], ms64[0:32, :], stqf[:])

        # ---- phase 2.5: qk-norm n2=0 (b0) + v(b0, heads 0-5), interleaved ----
        nc.sync.dma_start(wv_t[:], wv8.ap())
        for b in range(BPC):
            nc.vector.memset(v_sb[b][:], 0.0)
            nc.vector.memset(v_sb[b][:, :, :, 96:97], 1.0)

        def do_v(b, nk, h0, h1, pool):
            tsl = slice((b * 8 + nk) * 128, (b * 8 + nk + 1) * 128)
            nh = (h1 - h0) * HD
            vt = pool.tile([128, 2, 512], FP32, tag="fmm", name="fmm")
            for jj in range(FKP // 2):
                nc.tensor.matmul(vt[:, 0, 0:nh], xn8[:, 2 * jj:2 * jj + 2, tsl],
                                 wv_t[:, 2 * jj:2 * jj + 2, h0 * HD:h1 * HD],
                                 start=(jj == 0), stop=(jj == FKP // 2 - 1),
                                 perf_mode=DRM)
            nc.vector.tensor_scalar_mul(
                v_sb[b][:, nk, h0:h1, 0:HD],
                vt[:, 0, 0:nh].rearrange("p (h d) -> p h d", h=h1 - h0), CV)

        def do_norm_chunk(mb, nsl, pool, sb):
            fsl = slice(mb * 128, (mb + 1) * 128)
            rpz = pool.tile([128, 2, 512], FP32, tag="fmm", name="fmm")
            nc.tensor.matmul(rpz[:, 0, :], exr_t[:, fsl], stq_r[:, nsl],
                             start=True, stop=True)
            nc.tensor.matmul(rpz[:, 1, :], exr_t[:, fsl], stq_mr[:, nsl],
                             start=True, stop=True)
            tqc = sb.tile([128, 512], FP32, tag="tqc", name="tqc", bufs=2)
            nc.vector.tensor_tensor(tqc[:], qs8[:, mb, nsl], rpz[:, 0, :],
                                    op=ALU.mult)
            qn8c = sb.tile([128, 512], FP8, tag="qn8c", name="qn8c", bufs=2)
            nc.vector.tensor_tensor(qn8c[:], tqc[:], rpz[:, 1, :],
                                    op=ALU.subtract)
            nc.sync.dma_start(qk_d8[fsl, nsl], qn8c[:])

        nrm_order = [m for p in zip(range(FK), range(FK, QKF)) for m in p]
        with ExitStack() as ph:
            sb = ph.enter_context(tc.tile_pool(name="nrm", bufs=3))
            pe = ph.enter_context(tc.tile_pool(name="nrmpe", bufs=2, space="PSUM"))
            vix = 0
            for mb in nrm_order:
                for i in range(2):
                    do_norm_chunk(mb, slice(i * 512, (i + 1) * 512), pe, sb)
                if vix < 8:
                    do_v(0, vix, 0, 6, pe)
                    vix += 1

        # ------- merged region: attention + v rest + qk-norm(b1) + proj(b0) ---
        x2_p = None
        x2_sb = None  # allocated at b1 start, after front tiles release
        nc.vector.memset(o_sb[:, FK:FKP, :], 0.0)
        with ExitStack() as ph:
            qp = ph.enter_context(tc.tile_pool(name="aq", bufs=3,
                                               side="right"))
            up = ph.enter_context(tc.tile_pool(name="au", bufs=2,
                                               side="right"))
            ob = ph.enter_context(tc.tile_pool(name="ao", bufs=2,
                                               side="right"))
            wp = ph.enter_context(tc.tile_pool(name="pw", bufs=2,
                                               side="right"))
            sbp = ph.enter_context(tc.tile_pool(name="psb", bufs=3,
                                                side="right"))
            pss = ph.enter_context(tc.tile_pool(name="pss", bufs=2, space="PSUM"))
            pso = ph.enter_context(tc.tile_pool(name="pso", bufs=1, space="PSUM"))
            psz = ph.enter_context(tc.tile_pool(name="psz", bufs=1, space="PSUM"))
            pfl = ph.enter_context(tc.tile_pool(name="pfl", bufs=1, space="PSUM"))

            fillers = deque()

            def pop_filler():
                if fillers:
                    fillers.popleft()()

            for (h0, h1) in VBLK[1:]:
                for nk in range(8):
                    fillers.append(
                        lambda nk=nk, h0=h0, h1=h1: do_v(0, nk, h0, h1, pfl))
            for (h0, h1) in VBLK:
                for nk in range(8):
                    fillers.append(
                        lambda nk=nk, h0=h0, h1=h1: do_v(1, nk, h0, h1, pfl))
            for mb in nrm_order:
                def nrm_unit(mb=mb):
                    for i in range(2):
                        do_norm_chunk(mb, slice(1024 + i * 512,
                                                1024 + (i + 1) * 512), pfl, sbp)
                fillers.append(nrm_unit)

            # q/k tiles are [128, N] with the head dim zero-padded 72->128 so
            # the score matmuls use the full PE array (HAM counts them as
            # busy; 36-row DRM stationaries left the clock gate at K=4/8).
            def attn_scores(b, h, budget):
                c0 = b * N
                r0 = h * HD
                q_t = qp.tile([128, N], FP8, tag="qt", name="qt")
                nc.gpsimd.memset(q_t[64:128, :], 0.0)
                nc.sync.dma_start(q_t[0:HD, :], qk_d8[r0:r0 + HD, c0:c0 + N])
                k_t = qp.tile([128, N], FP8, tag="kt", name="kt")
                nc.gpsimd.memset(k_t[64:128, :], 0.0)
                nc.sync.dma_start(k_t[0:HD, :],
                                  qk_d8[H + r0:H + r0 + HD, c0:c0 + N])
                ut = up.tile([128, 8, N], FP8, tag="ut", name="ut")
                for nk in range(8):
                    ksl = slice(nk * 128, (nk + 1) * 128)
                    sp = pss.tile([128, 2, 512], FP32, tag="sp", name="sp")
                    for q2 in range(2):
                        nc.tensor.matmul(sp[:, q2, :], k_t[:, ksl],
                                         q_t[:, q2 * 512:(q2 + 1) * 512],
                                         start=True, stop=True)
                    nc.scalar.activation(ut[:, nk, :], sp[:, :, :], AF.Exp,
                                         bias=eps_t[:, 2:3], scale=CEXP)
                    if nk < budget:
                        pop_filler()
                return ut

            def attn_out(b, h, ut):
                c0 = b * N
                r0 = h * HD
                o8 = ob.tile([HD, N], FP8, tag="o8", name="o8")
                for q2 in range(2):
                    qsl = slice(q2 * 512, (q2 + 1) * 512)
                    avp = pso.tile([VCOL, 512], FP32, tag="avp", name="avp")
                    for jj in range(4):
                        nc.tensor.matmul(avp[:],
                                         v_sb[b][:, 2 * jj:2 * jj + 2, h, :],
                                         ut[:, 2 * jj:2 * jj + 2, qsl],
                                         start=(jj == 0), stop=(jj == 3),
                                         perf_mode=DRM)
                    zrow = ob.tile([1, 512], FP32, tag="zrow", name="zrow")
                    nc.vector.tensor_copy(zrow[:], avp[96:97, :])
                    rzf = ob.tile([1, 512], FP32, tag="rzf", name="rzf")
                    nc.vector.reciprocal_approx_fast(rzf[:], zrow[:])
                    rz = ob.tile([1, 512], BF16, tag="rz", name="rz")
                    nc.vector.tensor_copy(rz[:], rzf[:])
                    osb = ob.tile([HD, 512], FP32, tag="osb", name="osb")
                    nc.vector.tensor_copy(osb[:], avp[0:HD, :])
                    rzp = psz.tile([HD, 512], FP32, tag="rzp", name="rzp")
                    nc.tensor.matmul(rzp[:], ones16[0:1, 0:HD], rz[:],
                                     start=True, stop=True)
                    nc.vector.tensor_tensor(o8[:, qsl], osb[:], rzp[:],
                                            op=ALU.mult)
                c_, p_ = r0 // 128, r0 % 128
                n0 = min(HD, 128 - p_)
                nc.gpsimd.dma_start(o_sb[p_:p_ + n0, c_, c0:c0 + N], o8[0:n0, :])
                if n0 < HD:
                    nc.gpsimd.dma_start(o_sb[0:HD - n0, c_ + 1, c0:c0 + N],
                                        o8[n0:HD, :])

            def proj_unit(n, mb):
                def emit():
                    b = n // (NTC // BPC)
                    nsl = slice(n * 512, (n + 1) * 512)
                    msl = slice(mb * 128, (mb + 1) * 128)
                    wt = wp.tile([128, FKP, 128], FP8, tag="wpt", name="wpt")
                    nc.gpsimd.dma_start(
                        wt[:], wpro8.ap()[:, mb * FKP * 128:(mb + 1) * FKP * 128])
                    mm = pfl.tile([128, 2, 512], FP32, tag="fmm", name="fmm")
                    for jj in range(FKP // 2):
                        nc.tensor.matmul(mm[:, 0, :],
                                         wt[:, 2 * jj:2 * jj + 2, :],
                                         o_sb[:, 2 * jj:2 * jj + 2, nsl],
                                         start=(jj == 0),
                                         stop=(jj == FKP // 2 - 1),
                                         perf_mode=DRM)
                    xr = sbp.tile([128, 512], FP32R, tag="xr", name="xr", bufs=2)
                    nc.sync.dma_start(xr[:], xT.ap()[msl, nsl])
                    nc.vector.scalar_tensor_tensor(
                        x2_sb[:, mb, nsl], mm[:, 0, :],
                        g_col[0][b][:, mb:mb + 1], xr[:],
                        op0=ALU.mult, op1=ALU.add)
                return emit

            prev = None
            for b in range(BPC):
                for h in range(NH):
                    budget = min(8, -(-len(fillers) // max(1, NH - h)))
                    ut = attn_scores(b, h, budget)
                    if prev is not None:
                        attn_out(*prev)
                    if b == 1 and h == 0:
                        v_ps[0].release()
                        x2_p = tc.alloc_tile_pool(name="x2p", bufs=1)
                        x2_sb = x2_p.tile([128, FK, T], BF16, tag="x2",
                                          name="x2")
                        for n in (0, 1):
                            for mb in range(FK):
                                fillers.append(proj_unit(n, mb))
                    prev = (b, h, ut)
                if b == 0:
                    while fillers:
                        fillers.popleft()()
                    qs_p.release()
                    stq_p.release()
                    xn_p.release()
                    vw_p.release()
            attn_out(*prev)
            while fillers:
                fillers.popleft()()

        # ------- phase 6: proj(b1) + LN2 -> xn2; mlp1 -> h (fused) -------
        with ExitStack() as ph:
            sb = ph.enter_context(tc.tile_pool(name="l2sb", bufs=2))
            ro = ph.enter_context(tc.tile_pool(name="l2ro", bufs=1))
            stA16_2 = [ro.tile([1, 512], BF16, tag=f"sA{n}", name=f"sA{n}")
                       for n in range(NTC)]
            stC16_2 = [ro.tile([1, 512], BF16, tag=f"sC{n}", name=f"sC{n}")
                       for n in range(NTC)]
            src2 = lambda k: x2_sb[:, k, :]
            st_ps = ExitStack()
            ps = st_ps.enter_context(tc.tile_pool(name="l2ps", bufs=2,
                                                  space="PSUM"))
            pj = st_ps.enter_context(tc.tile_pool(name="l2pj", bufs=2,
                                                  space="PSUM"))
            pwp = st_ps.enter_context(tc.tile_pool(name="l2pw", bufs=2))
            pxr = st_ps.enter_context(tc.tile_pool(name="l2xr", bufs=2))

            def proj_mb(n, mb):
                b = n // (NTC // BPC)
                nsl = slice(n * 512, (n + 1) * 512)
                msl = slice(mb * 128, (mb + 1) * 128)
                wt = pwp.tile([128, FKP, 128], FP8, tag="wpt", name="wpt")
                nc.gpsimd.dma_start(
                    wt[:], wpro8.ap()[:, mb * FKP * 128:(mb + 1) * FKP * 128])
                mm = pj.tile([128, 512], FP32, tag="pjm", name="pjm")
                for jj in range(FKP // 2):
                    nc.tensor.matmul(mm[:], wt[:, 2 * jj:2 * jj + 2, :],
                                     o_sb[:, 2 * jj:2 * jj + 2, nsl],
                                     start=(jj == 0), stop=(jj == FKP // 2 - 1),
                                     perf_mode=DRM)
                xr = pxr.tile([128, 512], FP32R, tag="xr", name="xr")
                nc.sync.dma_start(xr[:], xT.ap()[msl, nsl])
                nc.vector.scalar_tensor_tensor(
                    x2_sb[:, mb, nsl], mm[:], g_col[0][b][:, mb:mb + 1], xr[:],
                    op0=ALU.mult, op1=ALU.add)

            for mb in range(FK):
                proj_mb(2, mb)
            ln_stats_n(src2, 0, ps, sb, ocol16, BF16, stA16_2, stC16_2)
            for mb in range(FK):
                proj_mb(3, mb)
            ln_stats_n(src2, 1, ps, sb, ocol16, BF16, stA16_2, stC16_2)
            ln_stats_n(src2, 2, ps, sb, ocol16, BF16, stA16_2, stC16_2)
            ln_stats_n(src2, 3, ps, sb, ocol16, BF16, stA16_2, stC16_2)
            st_ps.close()

            xn2_p = tc.alloc_tile_pool(name="xn2p", bufs=1, side="right")
            xn2 = xn2_p.tile([128, FKP, T], FP8, tag="xn2", name="xn2")
            nc.vector.memset(xn2[:, FK:FKP, :], 0.0)
            h_p = tc.alloc_tile_pool(name="hp", bufs=1, side="right")
            h_sb = h_p.tile([128, MK, T], FP8, tag="h_sb", name="h_sb")
            pe = ph.enter_context(tc.tile_pool(name="l2pe", bufs=1, space="PSUM"))
            wp = ph.enter_context(tc.tile_pool(name="m1w", bufs=2))
            msb = ph.enter_context(tc.tile_pool(name="m1sb", bufs=3,
                                                side="right"))
            mps = ph.enter_context(tc.tile_pool(name="m1ps", bufs=2, space="PSUM"))
            for n2 in range(BPC):
                ln_apply_b(src2, 1, xn2, sb, pe, stA16_2, stC16_2, n2)
                wsl = slice(n2 * 1024, (n2 + 1) * 1024)
                for mb in range(MK):
                    wt = wp.tile([128, FKP, 128], FP8, tag="wt", name="wt")
                    nc.sync.dma_start(
                        wt[:], w18.ap()[:, mb * FKP * 128:(mb + 1) * FKP * 128])
                    mm = mps.tile([128, 2, 512], FP32, tag="mm", name="mm")
                    for i in range(2):
                        nsl = slice((2 * n2 + i) * 512, (2 * n2 + i + 1) * 512)
                        for jj in range(FKP // 2):
                            nc.tensor.matmul(mm[:, i, :],
                                             wt[:, 2 * jj:2 * jj + 2, :],
                                             xn2[:, 2 * jj:2 * jj + 2, nsl],
                                             start=(jj == 0),
                                             stop=(jj == FKP // 2 - 1),
                                             perf_mode=DRM)
                    if SIM_COMPAT:
                        z = msb.tile([128, 1024], FP32, tag="gz", name="gz",
                                     bufs=1)
                        nc.scalar.activation(z[:], mm[:, :, :], AF.Identity,
                                             bias=0.0, scale=CG1)
                        t1 = msb.tile([128, 1024], FP32, tag="gt1", name="gt1",
                                      bufs=1)
                        nc.vector.tensor_mul(t1[:], z[:], z[:])
                        nc.vector.tensor_mul(t1[:], t1[:], z[:])
                        nc.vector.scalar_tensor_tensor(t1[:], t1[:], 0.044715,
                                                       z[:], op0=ALU.mult,
                                                       op1=ALU.add)
                        t4 = msb.tile([128, 1024], FP32, tag="gt4", name="gt4",
                                      bufs=1)
                        nc.scalar.activation(t4[:], t1[:], AF.Tanh, bias=0.0,
                                             scale=0.7978845608028654)
                        nc.vector.scalar_tensor_tensor(t4[:], t4[:], 1.0, z[:],
                                                       op0=ALU.add, op1=ALU.mult)
                        nc.vector.tensor_scalar_mul(h_sb[:, mb, wsl], t4[:], 0.5)
                    else:
                        nc.scalar.activation(h_sb[:, mb, wsl], mm[:, :, :],
                                             AF.Gelu_apprx_tanh, bias=0.0,
                                             scale=CG1)

        # ---------------- phase 7: mlp2 + gated residual -> out ----------------
        with ExitStack() as ph:
            wp = ph.enter_context(tc.tile_pool(name="m2w", bufs=2))
            sb = ph.enter_context(tc.tile_pool(name="m2sb", bufs=3))
            ps = ph.enter_context(tc.tile_pool(name="m2ps", bufs=2, space="PSUM"))
            for mb in range(FK):
                wt = wp.tile([128, MK, 128], FP8, tag="wt", name="wt")
                nc.sync.dma_start(
                    wt[:], w28.ap()[:, mb * MK * 128:(mb + 1) * MK * 128])
                msl = slice(mb * 128, (mb + 1) * 128)
                for b in range(BPC):
                    wsl = slice(b * N, (b + 1) * N)
                    mm = ps.tile([128, 2, 512], FP32, tag="mm", name="mm")
                    for i in range(2):
                        nsl = slice((2 * b + i) * 512, (2 * b + i + 1) * 512)
                        for jj in range(MK // 2):
                            nc.tensor.matmul(mm[:, i, :],
                                             wt[:, 2 * jj:2 * jj + 2, :],
                                             h_sb[:, 2 * jj:2 * jj + 2, nsl],
                                             start=(jj == 0),
                                             stop=(jj == MK // 2 - 1),
                                             perf_mode=DRM)
                    os_ = sb.tile([128, 1024], FP32, tag="os", name="os")
                    nc.vector.scalar_tensor_tensor(
                        os_[:], mm[:, :, :], g_col[1][b][:, mb:mb + 1],
                        x2_sb[:, mb, wsl], op0=ALU.mult, op1=ALU.add)
                    nc.sync.dma_start(out.ap()[msl, wsl], os_[:])
        h_p.release()
        xn2_p.release()
        x2_p.release()
        v_ps[1].release()
        o_p.release()

    nc.finalize()
    return nc


def _f8(x):
    return np.asarray(x, np.float32).astype(E4)


def _pack_dr(w, scale, kp, fblk=128):
    """[Kin, Mout] fp32 -> [128, (Mout//fblk)*kp*fblk] fp8, DR-stationary order
    [mb][plane][f]: pack[p, (mb*kp+pl)*fblk + f] = scale*w[pl*128+p, mb*fblk+f]."""
    kin, mout = w.shape
    wp = np.zeros((kp * 128, mout), np.float32)
    wp[:kin] = np.asarray(w, np.float32) * scale
    a = wp.reshape(kp, 128, mout // fblk, fblk)
    return _f8(np.ascontiguousarray(a.transpose(1, 2, 0, 3).reshape(128, -1)))


def _host_inputs(x, c, w_mod, b_mod, w_qkv, b_qkv, g_q, g_k, w_proj, b_proj,
                 w1, b1, w2, b2):
    f32 = np.float32
    w_qkv = np.asarray(w_qkv, f32)

    # v moving layout: [p, pl*H + col]
    wv = np.zeros((FKP * 128, H), f32)
    wv[:H] = w_qkv[:, 2 * H:] * SWV
    wv8m = _f8(wv.reshape(FKP, 128, H).transpose(1, 0, 2).reshape(128, -1))

    exr = np.zeros((32, QKF * 128), f32)
    gq = np.asarray(g_q, f32)
    gk = np.asarray(g_k, f32)
    for gf in range(2 * H):
        if gf < H:
            s, g = gf // HD, gq[gf % HD]
        else:
            s, g = 16 + (gf - H) // HD, gk[(gf - H) % HD]
        exr[s, gf] = SQN * g

    inds = np.zeros((128, QKF * 64), f32)
    indq = np.zeros((128, QKF * 64), f32)
    for mb in range(QKF):
        for f in range(128):
            gf = mb * 128 + f
            if gf < H:
                hh, base = gf // HD, 0
            else:
                hh, base = (gf - H) // HD, 16
            inds[f, mb * 64 + base + hh] = 1.0
            indq[f, mb * 64 + 32 + base + hh] = 1.0

    epsc = np.zeros((128, 4), f32)
    epsc[:, 0] = EPS
    epsc[:, 1] = EPS * SQ * SQ
    epsc[:, 2] = EXPB

    shared = {
        "wmod8": np.ascontiguousarray(
            np.asarray(w_mod, f32).reshape(FK, 128, 18, 384)
            .transpose(1, 2, 0, 3).reshape(128, -1)).astype(ml_dtypes.bfloat16),
        "bmod_s": (np.asarray(b_mod, f32).reshape(1, 6 * H)
                   * (SMOD * SSW)).astype(ml_dtypes.bfloat16),
        "wqk8": _pack_dr(np.ascontiguousarray(w_qkv[:, :2 * H]), SWQK, FKP),
        "wv8": wv8m,
        "bqk": np.ascontiguousarray(
            np.asarray(b_qkv, f32)[:2 * H].reshape(QKF, 128).T) * SQ,
        "exr": exr.astype(ml_dtypes.bfloat16),
        "ind8s": _f8(inds),
        "ind8q": _f8(indq),
        "wpro8": _pack_dr(np.asarray(w_proj, f32), SWPRO, FKP),
        "w18": _pack_dr(np.asarray(w1, f32), SW1, FKP),
        "w28": _pack_dr(np.asarray(w2, f32), SW2, MK),
        "onesr": np.ones((1, 512), f32),
        "onesb": np.ones((1, 512), f32).astype(ml_dtypes.bfloat16),
        "onesc": np.ones((128, 1), f32),
        "epsc": epsc,
    }

    in_maps = []
    for core in range(NCORES):
        xs = np.asarray(x[core * BPC:(core + 1) * BPC], f32)   # [2, N, H]
        m = dict(shared)
        m["xT"] = np.ascontiguousarray(xs.reshape(T, H).T)
        m["cT"] = np.ascontiguousarray(
            np.asarray(c[core * BPC:(core + 1) * BPC], f32).T)
        in_maps.append(m)
    return in_maps


def kernel(**inputs):
    if "nc" not in _CACHE:
        _CACHE["nc"] = _build_program()
    nc = _CACHE["nc"]
    in_maps = _host_inputs(**inputs)
    res = run_bass_kernel_spmd(nc, in_maps, core_ids=list(range(NCORES)))
    outs = [res.results[core]["out"].T.reshape(BPC, N, H) for core in range(NCORES)]
    return np.concatenate(outs, axis=0).astype(np.float32)
